# revision 22
# baseline (speedup 1.0000x reference)
"""Dense transformer (B=4,T=1024,C=1024,H=16,L=8) on 8 TRN2 NeuronCores.

Sharding: DP over batch (4) x sequence block-cyclic (2). Core c = 2b+s handles
batch b, token blocks {j : j%2==s} (128 tokens each, 512 tokens/core).

v2 design: instead of exchanging K/V per layer, the pair cores exchange the
fp8 NORMALIZED activations x8 = LN(h)*2^AX (one chunked 2-rank AllGather of
512KB per layer) and each core recomputes K/V for the full 1024 tokens from
the gathered x8. The AllGather output is rank-ordered == parity-ordered, so
all per-core data placement is compile-time identical (SPMD); only the causal
masks differ per core and ride in as runtime inputs (mask_a for parity-0 key
blocks, mask_b for parity-1).

Q/K/V and FC1 all consume pre-normalized fp8 activations, so PSUM evictions
are single scalar-engine ops (Copy / Gelu with a constant power-of-2 scale)
instead of vector-engine tensor_tensor chains - this keeps the tensor engine
fed so the HAM clock gate stays at 8/8 (2.4GHz). Activation-table switches
(Sqrt/Exp/Gelu live in different tables, 1.28us reload) are prefetched with
dummy [128,1] ops in scalar-engine idle windows.

fp8: the weight-stationary GEMMs run fp8e4m3 MatmulPerfMode.DoubleRow
(pairs of 128-channel k-tiles per instruction). Weights scaled by per-class
powers of two; descales fold into Exp's scale, the softmax-denominator
path, the Gelu eviction scale, and scalar_tensor_tensor mult slots.
Attention (QK/AV) and the head stay bf16; residual h is f32r; all matmuls
accumulate in fp32 PSUM. MLP of layers in MLP8 runs fp8; the rest bf16.
"""
import sys
import os
import numpy as np
import ml_dtypes

sys.path.insert(0, '/opt/trn_rl_repo')

import concourse.bass as bass  # noqa: F401
import concourse.tile as tile
from concourse import bacc, mybir
from concourse.bass_utils import run_bass_kernel_spmd

F = mybir.ActivationFunctionType
ALU = mybir.AluOpType
PM = mybir.MatmulPerfMode
dt = mybir.dt
AF32 = np.float32
ABF16 = ml_dtypes.bfloat16
AE4 = ml_dtypes.float8_e4m3fn

B, T, C, H, L = 4, 1024, 1024, 16, 8
DIN, DOUT, DH = 128, 256, 64
TL = 512
NB = 4
NCH = C // 128
NPR = NCH // 2
DFF = 4 * C
EPS = 1e-5

_CACHE = {}
L_RUN = int(os.environ.get('KLAYERS', str(L)))

# activation pre-scale exponents (pow2) applied before fp8 casts to keep
# small values out of e4m3 subnormal range; descale folds into existing
# eviction constants.
AX = 3   # normalized x8 (QKV stationary input)
AY = 5   # attention output y8

# layers whose MLP runs fp8 DoubleRow; the rest run bf16. fp8 MLP noise
# dominates the error budget, and EARLY layer noise is ~4x more damaging
# than late (it compounds through the stack) - so late layers go fp8.
MLP8 = frozenset(
    int(x) for x in os.environ.get('KMLP8', '3,4,5,6,7').split(',') if x != '')


def _build(use_bias, sq, sk, sv, so, s1, s2, mlp8):
    """use_bias: dict of bools; s*: power-of-2 exponents of the fp8 weight
    scaling per matrix class (weights stored as W*2^s); mlp8: layer indices
    whose MLP runs fp8 DoubleRow (others run bf16 for accuracy)."""
    nc = bacc.Bacc("TRN2", target_bir_lowering=False, debug=False, num_devices=8)
    l16 = sorted(set(range(L)) - set(mlp8))
    l16idx = {li: i for i, li in enumerate(l16)}

    def par(name, shape, dtp):
        return nc.declare_dram_parameter(name, list(shape), dtp, isOutput=False)

    xloc = par('xloc', [128, TL], dt.bfloat16)
    xshift = par('xshift', [128, TL], dt.float32)
    emb_w = par('emb_w', [128, C], dt.bfloat16)
    emb_b = par('emb_b', [128, NCH], dt.float32)
    wq = par('wq', [L, NPR, 128, 2, C], dt.float8e4)
    wk = par('wk', [L, NPR, 128, 2, C], dt.float8e4)
    wv = par('wv', [L, NPR, 128, 2, C], dt.float8e4)
    bq = par('bq', [L, 128, NCH], dt.float32)
    bk = par('bk', [L, 128, NCH], dt.float32)
    bv = par('bv', [L, 128, NCH], dt.float32)
    wo = par('wo', [L, NPR, 128, 2, C], dt.float8e4)
    bo = par('bo', [L, 128, NCH], dt.float32)
    w1 = par('w1', [L, NPR, 4, 128, 2, 1024], dt.float8e4)
    b1 = par('b1', [L, 128, 32], dt.float32)
    w2 = par('w2', [L, DFF // 256, 128, 2, C], dt.float8e4)
    b2 = par('b2', [L, 128, NCH], dt.float32)
    if l16:
        w1b = par('w1b', [len(l16), C, DFF], dt.bfloat16)
        w2b = par('w2b', [len(l16), DFF, C], dt.bfloat16)
    pw = par('pw', [C, DOUT], dt.bfloat16)
    pb = par('pb', [128, 2], dt.float32)
    dw1 = par('dw1', [DOUT, DOUT], dt.bfloat16)
    db1 = par('db1', [128, 2], dt.float32)
    dw2 = par('dw2', [DOUT, DIN], dt.bfloat16)
    db2 = par('db2', [128, 1], dt.float32)
    mask_a = par('mask_a', [128, 128], dt.bfloat16)
    mask_b = par('mask_b', [128, 128], dt.bfloat16)
    out_p = nc.declare_dram_parameter('out', [128, TL], dt.float32, isOutput=True)

    exp_scale = float(2.0 ** (-(sq + sk + 3 * AX)))

    with tile.TileContext(nc, num_cores=8) as tc:
        with tc.tile_pool(name='persist', bufs=1) as pp, \
             tc.tile_pool(name='sbwork', bufs=1) as wkp, \
             tc.tile_pool(name='wslab', bufs=1) as wsp, \
             tc.tile_pool(name='small', bufs=1) as smp, \
             tc.tile_pool(name='dram', bufs=2, space='DRAM') as drp:

            hT = pp.tile([128, NCH, TL], dt.float32r, name='hT')
            h8 = pp.tile([128, NCH, TL], dt.float8e4, name='h8')
            x8l = pp.tile([128, NCH, TL], dt.float8e4, name='x8l')
            QT = pp.tile([128, NCH, TL], dt.bfloat16, name='QT')
            # K for full T, parity-indexed: [:, c, par, t]
            KT = pp.tile([128, NCH, 2, TL], dt.bfloat16, name='KT')
            # V stored head-group-major [g, par, j, hh, 128]: cols 0:64 are V
            # data, cols 64:128 are ones so the AV matmul emits the softmax
            # denominator replicated on PSUM partitions 64:128.
            Vv = pp.tile([128, 4, 2, NB, 4, 128], dt.bfloat16, name='Vv')
            y8 = pp.tile([128, NCH, TL], dt.float8e4, name='y8')
            mska = pp.tile([128, 128], dt.bfloat16, name='mska')
            mskb = pp.tile([128, 128], dt.bfloat16, name='mskb')
            ones_mf = pp.tile([128, 128], dt.float32, name='ones_mf')
            ones_mat = pp.tile([128, 128], dt.float32r, name='ones_mat')
            ones_f32 = pp.tile([128, 1], dt.float32, name='ones_f32')
            eps_t = pp.tile([128, 1], dt.float32, name='eps_t')
            dscO = pp.tile([128, 1], dt.float32, name='dscO')
            dscW2 = pp.tile([128, 1], dt.float32, name='dscW2')
            cmAX = pp.tile([128, 1], dt.float32, name='cmAX')
            cLnAX = pp.tile([128, 1], dt.float32, name='cLnAX')
            dum = pp.tile([128, 1], dt.float32, name='dum')
            xl_sb = pp.tile([128, TL], dt.bfloat16, name='xl_sb')
            xsh_sb = pp.tile([128, TL], dt.float32, name='xsh_sb')
            featsT = pp.tile([128, 2, TL], dt.bfloat16, name='featsT')
            zT = pp.tile([128, 2, TL], dt.float32r, name='zT')
            out_sb = pp.tile([128, TL], dt.float32, name='out_sb')

            nc.sync.dma_start(mska[:], mask_a[:])
            nc.sync.dma_start(mskb[:], mask_b[:])
            nc.sync.dma_start(xl_sb[:], xloc[:])
            nc.sync.dma_start(xsh_sb[:], xshift[:])
            nc.vector.memset(ones_f32[:], 1.0)
            nc.vector.memset(ones_mf[:], 1.0)
            nc.vector.tensor_copy(ones_mat[:], ones_mf[:])
            nc.vector.memset(eps_t[:], EPS)
            nc.vector.memset(dscO[:], float(2.0 ** (-(so + AY))))
            nc.vector.memset(dscW2[:], float(2.0 ** (-s2)))
            nc.vector.memset(cmAX[:], float(2.0 ** (-AX)))
            nc.vector.memset(cLnAX[:], float(AX * np.log(2.0)))
            nc.vector.memset(Vv[:, :, :, :, :, 64:128], 1.0)

            psA = None

            LN2 = float(np.log(2.0))

            def ln_scale(src, nch, sexp=0):
                """[128, TL] f32 of rstd*2^sexp for src [128, nch, TL].
                Stats are computed replicated on all 128 partitions (ones
                matrix as matmul lhsT). rstd = exp(-0.5*ln(var+eps)): Ln and
                Exp share one activation table with attention's Exp, so no
                Sqrt table ever loads (saves 2 x 1.28us table swaps/layer)."""
                ps_su = psA.tile([128, TL], dt.float32, tag='ps', bufs=8,
                                 name='ps_su')
                for c in range(nch):
                    nc.tensor.matmul(ps_su[:], ones_mat[:], src[:, c, :],
                                     start=(c == 0), stop=(c == nch - 1))
                mu = smp.tile([128, TL], dt.float32, tag='lnmu', bufs=1,
                              name='mu')
                nc.scalar.activation(mu[:], ps_su[:], F.Copy,
                                     scale=1.0 / (nch * 128))
                ps_sq = psA.tile([128, TL], dt.float32, tag='ps', bufs=8,
                                 name='ps_sq')
                for c in range(nch):
                    sq_t = wkp.tile([128, TL], dt.float32r, tag='sq', bufs=2,
                                    name='sq')
                    nc.scalar.activation(sq_t[:], src[:, c, :], F.Square)
                    nc.tensor.matmul(ps_sq[:], ones_mat[:], sq_t[:],
                                     start=(c == 0), stop=(c == nch - 1))
                var = smp.tile([128, TL], dt.float32, tag='lnvar', bufs=1,
                               name='var')
                nc.scalar.activation(var[:], ps_sq[:], F.Copy,
                                     scale=1.0 / (nch * 128), bias=EPS)
                scr = smp.tile([128, TL], dt.float32, tag='lnscr', bufs=1,
                               name='scr')
                nc.vector.tensor_mul(scr[:], mu[:], mu[:])
                nc.vector.tensor_sub(var[:], var[:], scr[:])
                lv = smp.tile([128, TL], dt.float32, tag='lnlv', bufs=1,
                              name='lv')
                nc.scalar.activation(lv[:], var[:], F.Ln)
                sb = wkp.tile([128, TL], dt.float32, tag='lnsb', bufs=2,
                              name='sb')
                nc.scalar.activation(sb[:], lv[:], F.Exp, scale=-0.5,
                                     bias=(cLnAX[:, 0:1] if sexp else 0.0))
                return sb

            def prefetch(fn):
                # dummy [128,1] activation: loads fn's table while the
                # scalar engine is otherwise idle, off the critical path
                nc.scalar.activation(dum[:], eps_t[:], fn)

            def matphase8(src_sel, w_ap, l, npr_in, nch_out, out_cb, halves=1):
                """fp8 DoubleRow: out[co] = sum_p pair(w).T @ pair(src).
                src_sel(p) gives the [128, 2, TL] fp8 rhs for k-pair p.
                halves=2 splits the output channels into two 4-bank PSUM
                groups so the phase starts while the previous phase's other
                banks still drain."""
                nh = nch_out // halves
                for hh in range(halves):
                    pss = [psA.tile([128, TL], dt.float32, tag='ps', bufs=8,
                                    name=f'pp{co}') for co in range(nh)]
                    for p in range(npr_in):
                        slab = wsp.tile([128, 2, nh * 128], dt.float8e4,
                                        tag='wslab8', bufs=8, name='slab')
                        wsl = w_ap[l, p] if l is not None else w_ap[p]
                        nc.sync.dma_start(
                            slab[:], wsl[:, :, hh * nh * 128:(hh + 1) * nh * 128])
                        for co in range(nh):
                            nc.tensor.matmul(pss[co][:],
                                             slab[:, :, co * 128:(co + 1) * 128],
                                             src_sel(p),
                                             start=(p == 0),
                                             stop=(p == npr_in - 1),
                                             perf_mode=PM.DoubleRow)
                    for co in range(nh):
                        out_cb(hh * nh + co, pss[co])

            def matphase(src, w_ap, l, nch_in, nch_out, out_cb, wtag, wdt):
                """bf16: out[co] = sum_ci w[ci,co].T @ src[:,ci,:]."""
                pss = [psA.tile([128, TL], dt.float32, tag='ps', bufs=8,
                                name=f'pp{co}') for co in range(nch_out)]
                for ci in range(nch_in):
                    slab = wsp.tile([128, nch_out * 128], wdt, tag=wtag,
                                    bufs=(4 if wtag == 'wslab' else 5), name='slab')
                    src_w = w_ap[l, ci * 128:(ci + 1) * 128, :] if l is not None \
                        else w_ap[ci * 128:(ci + 1) * 128, :]
                    nc.sync.dma_start(slab[:], src_w)
                    for co in range(nch_out):
                        nc.tensor.matmul(pss[co][:], slab[:, co * 128:(co + 1) * 128],
                                         src[:, ci, :], start=(ci == 0),
                                         stop=(ci == nch_in - 1))
                for co in range(nch_out):
                    out_cb(co, pss[co])

            # ---------------- embed ----------------
            with tc.tile_pool(name='psE', bufs=1, space='PSUM') as psA:
                embs = wsp.tile([128, C], dt.bfloat16, tag='wslabb', bufs=5,
                                name='embs')
                nc.sync.dma_start(embs[:], emb_w[:])
                ebias = smp.tile([128, NCH], dt.float32, tag='bias8', bufs=2,
                                 name='ebias')
                nc.sync.dma_start(ebias[:], emb_b[:])
                for co in range(NCH):
                    ps = psA.tile([128, TL], dt.float32, tag='ps', bufs=8,
                                  name=f'pe{co}')
                    nc.tensor.matmul(ps[:], embs[:, co * 128:(co + 1) * 128],
                                     xl_sb[:], start=True, stop=True)
                    nc.scalar.activation(hT[:, co, :], ps[:], F.Identity,
                                         bias=ebias[:, co:co + 1])

            # ---------------- layers ----------------
            for li in range(L_RUN):
                with tc.tile_pool(name=f'psA{li}', bufs=1, space='PSUM') as psA:
                    # x8f shares one ring slot with the MLP input (m8/m16):
                    # their lifetimes are disjoint within a layer.
                    x8f = wkp.tile([128, NCH, 2, TL], dt.float8e4, tag='xph',
                                   bufs=1, name='x8f')
                    # h8 = h*2^AX needs no LN stats - it feeds the Q
                    # projection, whose per-token rstd factors out of the
                    # matmul and rides on the eviction. Those Q matmuls keep
                    # the PE busy through the LN tail AND the x8 AllGather
                    # latency (K/V need the gathered x8).
                    for cc in range(NCH):
                        nc.vector.tensor_scalar_mul(h8[:, cc, :], hT[:, cc, :],
                                                    float(2.0 ** AX))
                    sb1 = ln_scale(hT, NCH, sexp=AX)

                    # x8 = LN(h)*2^AX; AllGather launched per channel-pair
                    # chunk so K/V-proj accumulation can start on chunk 0
                    # while later chunks are still in flight.
                    outbs = []
                    for ch in range(NPR):
                        for cc in (2 * ch, 2 * ch + 1):
                            nc.vector.tensor_mul(x8l[:, cc, :], hT[:, cc, :],
                                                 sb1[:])
                        inb = drp.tile([128, 2 * TL], dt.float8e4,
                                       tag='inbX', bufs=8, name='inbX')
                        outb = drp.tile([256, 2 * TL], dt.float8e4,
                                        tag='outbX', bufs=8, name='outbX')
                        nc.sync.dma_start(
                            inb[:], x8l[:, 2 * ch:2 * ch + 2, :])
                        nc.gpsimd.collective_compute(
                            "AllGather", ALU.bypass,
                            replica_groups=[[0, 1], [2, 3], [4, 5], [6, 7]],
                            ins=[inb.opt()], outs=[outb.opt()])
                        outbs.append(outb)

                    qb = smp.tile([128, NCH], dt.float32, tag='bias8', bufs=2,
                                  name='qb')
                    if use_bias['qkv']:
                        nc.sync.dma_start(qb[:], bq[li])

                    def evict_q(co, ps):
                        nc.vector.tensor_mul(QT[:, co, :], ps[:], sb1[:])
                        if use_bias['qkv']:
                            nc.vector.tensor_scalar_add(
                                QT[:, co, :], QT[:, co, :], qb[:, co:co + 1])
                    matphase8(lambda p: h8[:, 2 * p:2 * p + 2, :],
                              wq, li, NPR, NCH, evict_q, halves=2)

                    # gathered x8 chunks land parity-indexed (AllGather
                    # output is rank-ordered and rank pair index == token
                    # parity), so placement is identical on every core.
                    for ch in range(NPR):
                        for par in range(2):
                            nc.sync.dma_start(
                                x8f[:, 2 * ch:2 * ch + 2, par, :],
                                outbs[ch][128 * par:128 * par + 128, :])

                    kb = smp.tile([128, NCH], dt.float32, tag='bias8', bufs=2,
                                  name='kb')
                    if use_bias['qkv']:
                        nc.sync.dma_start(kb[:], bk[li])

                    for kpar in range(2):
                        def evict_k(co, ps, kpar=kpar):
                            if use_bias['qkv']:
                                nc.scalar.activation(KT[:, co, kpar, :], ps[:],
                                                     F.Identity,
                                                     bias=kb[:, co:co + 1])
                            else:
                                nc.scalar.activation(KT[:, co, kpar, :], ps[:],
                                                     F.Copy)
                        matphase8(lambda p, kpar=kpar:
                                  x8f[:, 2 * p:2 * p + 2, kpar, :],
                                  wk, li, NPR, NCH, evict_k, halves=2)

                    # V proj (token-major): psv[tb] = x8f[:,:,par,tb].T @ Wv
                    # half. Weight slabs hoisted and reused across the 4
                    # (par, dvh) quarter-phases; 4 PSUM banks per phase.
                    vslabs = []
                    for p in range(NPR):
                        vs = wsp.tile([128, 2, C], dt.float8e4, tag='vslab',
                                      bufs=4, name='vslab')
                        nc.sync.dma_start(vs[:], wv[li, p])
                        vslabs.append(vs)
                    for vpar in range(2):
                        for dvh in range(2):
                            psv = [psA.tile([128, TL], dt.float32, tag='ps',
                                            bufs=8, name=f'pv{i}')
                                   for i in range(NB)]
                            for p in range(NPR):
                                for tb in range(NB):
                                    nc.tensor.matmul(
                                        psv[tb][:],
                                        x8f[:, 2 * p:2 * p + 2, vpar,
                                            tb * 128:(tb + 1) * 128],
                                        vslabs[p][:, :, dvh * 512:(dvh + 1) * 512],
                                        start=(p == 0), stop=(p == NPR - 1),
                                        perf_mode=PM.DoubleRow)
                            for tb in range(NB):
                                src = psv[tb][:].rearrange(
                                    "p (h e) -> p h e", e=64)
                                for gg in range(2):
                                    dst = Vv[:, 2 * dvh + gg, vpar, tb, 0:4, 0:64]
                                    nc.scalar.activation(
                                        dst, src[:, 4 * gg:4 * gg + 4, :], F.Copy)

                with tc.tile_pool(name=f'psB{li}', bufs=1, space='PSUM') as psB:
                    vbl = smp.tile([128, NCH], dt.float32, tag='bias8v', bufs=2,
                                   name='vbl')
                    if use_bias['v']:
                        nc.sync.dma_start(vbl[:], bv[li])

                    def att_pass(cp, psy, apar):
                        msk = mska if apar == 0 else mskb
                        for j in range(NB):
                            qs = 128 * j
                            qn = TL - qs
                            pssc = psB.tile([128, 2, TL], dt.float32, tag='pssc',
                                            bufs=2, name='pssc')
                            for hp in range(2):
                                nc.tensor.matmul(
                                    pssc[:, hp, 0:qn],
                                    KT[hp * 64:(hp + 1) * 64, cp, apar,
                                       j * 128:(j + 1) * 128],
                                    QT[hp * 64:(hp + 1) * 64, cp, qs:TL],
                                    start=True, stop=True)
                            et = wkp.tile([128, 2, qn], dt.bfloat16, tag='et',
                                          bufs=3, name='et')
                            nc.scalar.activation(et[:], pssc[:, :, 0:qn], F.Exp,
                                                 scale=exp_scale)
                            for hp in range(2):
                                nc.vector.tensor_mul(et[:, hp, 0:128],
                                                     et[:, hp, 0:128], msk[:])
                            for hp in range(2):
                                nc.tensor.matmul(
                                    psy[:, hp, qs:TL],
                                    Vv[:, cp // 2, apar, j, 2 * (cp % 2) + hp, :],
                                    et[:, hp, :],
                                    start=(apar == 0 and j == 0),
                                    stop=(apar == 1 and j == NB - 1))

                    def att_evict(cp, psy):
                        # fold V's 2^(sv+AX) and y8's 2^AY prescale into the
                        # numerator copy; psy[64:128] is the softmax
                        # denominator replicated across 64 partitions.
                        ysb = wkp.tile([64, 2, TL], dt.bfloat16, tag='ysb', bufs=1,
                                       name='ysb')
                        nc.scalar.activation(ysb[:], psy[0:64, :, :], F.Copy,
                                             scale=float(2.0 ** (AY - sv - AX)))
                        rbd = wkp.tile([64, 2, TL], dt.float32, tag='rbd',
                                       bufs=1, name='rbd')
                        nc.scalar.activation(rbd[:], psy[64:128, :, :], F.Copy)
                        rb = wkp.tile([64, 2, TL], dt.float32, tag='rb', bufs=1,
                                      name='rb')
                        nc.vector.reciprocal_approx_fast(rb[:], rbd[:])
                        for hp in range(2):
                            nc.vector.tensor_mul(y8[hp * 64:(hp + 1) * 64, cp, :],
                                                 ysb[:, hp, :], rb[:, hp, :])
                            if use_bias['v']:
                                nc.vector.tensor_scalar_add(
                                    y8[hp * 64:(hp + 1) * 64, cp, :],
                                    y8[hp * 64:(hp + 1) * 64, cp, :],
                                    vbl[hp * 64:(hp + 1) * 64, cp:cp + 1])

                    for cw in range(4):
                        for ci in range(2):
                            cp = 2 * cw + ci
                            psy = psB.tile([128, 2, TL], dt.float32, tag='psy',
                                           bufs=2, name='psy')
                            att_pass(cp, psy, 0)
                            att_pass(cp, psy, 1)
                            att_evict(cp, psy)

                with tc.tile_pool(name=f'psC{li}', bufs=1, space='PSUM') as psA:
                    obias = smp.tile([128, NCH], dt.float32, tag='bias8o',
                                     bufs=2, name='obias')
                    if use_bias['o']:
                        nc.sync.dma_start(obias[:], bo[li])

                    def evict_proj(co, ps):
                        nc.vector.scalar_tensor_tensor(
                            hT[:, co, :], ps[:], dscO[:, 0:1], hT[:, co, :],
                            ALU.mult, ALU.add)
                        if use_bias['o']:
                            nc.vector.tensor_scalar_add(
                                hT[:, co, :], hT[:, co, :], obias[:, co:co + 1])
                    matphase8(lambda p: y8[:, 2 * p:2 * p + 2, :],
                              wo, li, NPR, NCH, evict_proj, halves=2)

                    b1s = smp.tile([128, 32], dt.float32, tag='b1s', bufs=2,
                                   name='b1s')
                    if use_bias['fc1']:
                        nc.sync.dma_start(b1s[:], b1[li])
                    b2s = smp.tile([128, NCH], dt.float32, tag='bias8', bufs=2,
                                   name='b2s')
                    if use_bias['fc2']:
                        nc.sync.dma_start(b2s[:], b2[li])

                    sb2 = ln_scale(hT, NCH, sexp=AX)
                    prefetch(F.Gelu)

                    if li in mlp8:
                        g1sc = float(2.0 ** (-(s1 + AX)))
                        m8 = wkp.tile([128, NCH, TL], dt.float8e4, tag='xph',
                                      bufs=1, name='m8')
                        for cc in range(NCH):
                            nc.vector.tensor_mul(m8[:, cc, :], hT[:, cc, :],
                                                 sb2[:])
                        m_act = wkp.tile([128, 32, TL], dt.float8e4, tag='mact',
                                         bufs=1, name='m_act8')
                        for fog in range(4):
                            psf = [psA.tile([128, TL], dt.float32, tag='ps',
                                            bufs=8, name=f'pf{i}')
                                   for i in range(8)]
                            for p in range(NPR):
                                slab = wsp.tile([128, 2, 1024], dt.float8e4,
                                                tag='wslab8', bufs=8,
                                                name='f1slab')
                                nc.sync.dma_start(slab[:], w1[li, p, fog])
                                for fo in range(8):
                                    nc.tensor.matmul(
                                        psf[fo][:],
                                        slab[:, :, fo * 128:(fo + 1) * 128],
                                        m8[:, 2 * p:2 * p + 2, :],
                                        start=(p == 0), stop=(p == NPR - 1),
                                        perf_mode=PM.DoubleRow)
                            for fo in range(8):
                                fi = fog * 8 + fo
                                nc.scalar.activation(
                                    m_act[:, fi, :], psf[fo][:], F.Gelu,
                                    bias=(b1s[:, fi:fi + 1]
                                          if use_bias['fc1'] else 0.0),
                                    scale=g1sc)
                        for coh in range(2):
                            psm = [psA.tile([128, TL], dt.float32, tag='ps',
                                            bufs=8, name=f'pm{i}')
                                   for i in range(4)]
                            for p in range(16):
                                slab = wsp.tile([128, 2, C // 2], dt.float8e4,
                                                tag='wslab8', bufs=8,
                                                name='f2slab')
                                nc.sync.dma_start(
                                    slab[:],
                                    w2[li, p, :, :, coh * 512:(coh + 1) * 512])
                                for c4 in range(4):
                                    nc.tensor.matmul(
                                        psm[c4][:],
                                        slab[:, :, c4 * 128:(c4 + 1) * 128],
                                        m_act[:, 2 * p:2 * p + 2, :],
                                        start=(p == 0), stop=(p == 15),
                                        perf_mode=PM.DoubleRow)
                            for c4 in range(4):
                                co = coh * 4 + c4
                                nc.vector.scalar_tensor_tensor(
                                    hT[:, co, :], psm[c4][:], dscW2[:, 0:1],
                                    hT[:, co, :], ALU.mult, ALU.add)
                                if use_bias['fc2']:
                                    nc.vector.tensor_scalar_add(
                                        hT[:, co, :], hT[:, co, :],
                                        b2s[:, co:co + 1])
                    else:
                        # bf16 MLP layer (precision recovery): normalized
                        # input in bf16, gelu straight off PSUM.
                        m16 = wkp.tile([128, NCH, TL], dt.bfloat16, tag='xph',
                                       bufs=1, name='m16')
                        for cc in range(NCH):
                            nc.vector.scalar_tensor_tensor(
                                m16[:, cc, :], hT[:, cc, :], cmAX[:, 0:1],
                                sb2[:], ALU.mult, ALU.mult)
                        m16a = wkp.tile([128, 32, TL], dt.bfloat16, tag='mact',
                                        bufs=1, name='m16a')
                        lb = l16idx[li]
                        for fog in range(4):
                            psf = [psA.tile([128, TL], dt.float32, tag='ps',
                                            bufs=8, name=f'pf{i}')
                                   for i in range(8)]
                            for ci in range(NCH):
                                slab = wsp.tile([128, C], dt.bfloat16,
                                                tag='wslabb', bufs=5,
                                                name='f1slabb')
                                nc.sync.dma_start(
                                    slab[:], w1b[lb, ci * 128:(ci + 1) * 128,
                                                 fog * 1024:(fog + 1) * 1024])
                                for fo in range(8):
                                    nc.tensor.matmul(
                                        psf[fo][:],
                                        slab[:, fo * 128:(fo + 1) * 128],
                                        m16[:, ci, :], start=(ci == 0),
                                        stop=(ci == NCH - 1))
                            for fo in range(8):
                                fi = fog * 8 + fo
                                nc.scalar.activation(
                                    m16a[:, fi, :], psf[fo][:], F.Gelu,
                                    bias=(b1s[:, fi:fi + 1]
                                          if use_bias['fc1'] else 0.0))
                        for coh in range(2):
                            psm = [psA.tile([128, TL], dt.float32, tag='ps',
                                            bufs=8, name=f'pm{i}')
                                   for i in range(4)]
                            for fi in range(32):
                                slab = wsp.tile([128, C // 2], dt.bfloat16,
                                                tag='wslabb', bufs=5,
                                                name='f2slabb')
                                nc.sync.dma_start(
                                    slab[:], w2b[lb, fi * 128:(fi + 1) * 128,
                                                 coh * 512:(coh + 1) * 512])
                                for c4 in range(4):
                                    nc.tensor.matmul(
                                        psm[c4][:],
                                        slab[:, c4 * 128:(c4 + 1) * 128],
                                        m16a[:, fi, :], start=(fi == 0),
                                        stop=(fi == 31))
                            for c4 in range(4):
                                co = coh * 4 + c4
                                nc.vector.scalar_tensor_tensor(
                                    hT[:, co, :], psm[c4][:], ones_f32[:, 0:1],
                                    hT[:, co, :], ALU.mult, ALU.add)
                                if use_bias['fc2']:
                                    nc.vector.tensor_scalar_add(
                                        hT[:, co, :], hT[:, co, :],
                                        b2s[:, co:co + 1])

            # ---------------- head ----------------
            with tc.tile_pool(name='psH', bufs=1, space='PSUM') as psA:
                hTb = wkp.tile([128, NCH, TL], dt.bfloat16, tag='mact', bufs=1,
                               name='hTb')
                for co in range(NCH):
                    nc.vector.tensor_copy(hTb[:, co, :], hT[:, co, :])
                sbf = ln_scale(hT, NCH)
                pbias = smp.tile([128, 2], dt.float32, tag='bias2', bufs=2,
                                 name='pbias')
                if use_bias['pw']:
                    nc.sync.dma_start(pbias[:], pb[:])

                def evict_pw(co, ps):
                    nc.vector.tensor_mul(featsT[:, co, :], ps[:], sbf[:])
                    if use_bias['pw']:
                        nc.vector.tensor_scalar_add(featsT[:, co, :],
                                                    featsT[:, co, :],
                                                    pbias[:, co:co + 1])
                matphase(hTb, pw, None, NCH, 2, evict_pw, 'wslabb', dt.bfloat16)

                d1b = smp.tile([128, 2], dt.float32, tag='bias2', bufs=2, name='d1b')
                if use_bias['dw1']:
                    nc.sync.dma_start(d1b[:], db1[:])

                def evict_d1(co, ps):
                    nc.scalar.activation(zT[:, co, :], ps[:], F.Tanh,
                                         bias=(d1b[:, co:co + 1]
                                               if use_bias['dw1'] else 0.0))
                matphase(featsT, dw1, None, 2, 2, evict_d1, 'wslabb', dt.bfloat16)

                sbz = ln_scale(zT, 2)
                zb = wkp.tile([128, 2, TL], dt.bfloat16, tag='zb', bufs=1, name='zb')
                for co in range(2):
                    nc.vector.tensor_copy(zb[:, co, :], zT[:, co, :])
                d2b = smp.tile([128, 1], dt.float32, tag='bias2', bufs=2, name='d2b')
                if use_bias['dw2']:
                    nc.sync.dma_start(d2b[:], db2[:])

                def evict_out(co, ps):
                    ptmp = wkp.tile([128, TL], dt.float32, tag='ptmp', bufs=1,
                                    name='ptmp')
                    nc.vector.tensor_mul(ptmp[:], ps[:], sbz[:])
                    if use_bias['dw2']:
                        nc.vector.scalar_tensor_tensor(out_sb[:], ptmp[:],
                                                       d2b[:, 0:1], xsh_sb[:],
                                                       ALU.add, ALU.subtract)
                    else:
                        nc.vector.tensor_sub(out_sb[:], ptmp[:], xsh_sb[:])
                matphase(zb, dw2, None, 2, 1, evict_out, 'wslabb', dt.bfloat16)
                nc.sync.dma_start(out_p[:], out_sb[:])

    nc.compile()
    return nc


def _fold(g, b, W, bias, scl=1.0):
    """LN(x;g,b) @ W + bias == (x @ W'')*rstd + v with the mean folded in."""
    g = np.asarray(g, np.float64)
    W = np.asarray(W, np.float64)
    u = g @ W
    Wf = (g[:, None] * W - u[None, :] / W.shape[0]) * scl
    v = (np.asarray(b, np.float64) @ W + np.asarray(bias, np.float64)) * scl
    return Wf.astype(AF32), v.astype(AF32)


def _r8(v):
    return np.ascontiguousarray(np.asarray(v, AF32).reshape(-1, 128).T)


def _sexp(absmax):
    """Largest s with absmax * 2^s <= 240."""
    return int(np.floor(np.log2(240.0 / max(absmax, 1e-30))))


def _pack8(W, s):
    """[Cin, Cout] f32 -> [Cin//256, 128, 2, Cout] fp8e4 scaled by 2^s."""
    cin, cout = W.shape
    Wr = (np.asarray(W, AF32) * np.float32(2.0 ** s)).reshape(
        cin // 256, 2, 128, cout).transpose(0, 2, 1, 3)
    return np.ascontiguousarray(Wr).astype(AE4)


def kernel(**inputs):
    x = np.asarray(inputs['tokens'], AF32).reshape(B, T, DIN)
    ln1_g, ln1_b = inputs['ln1_g'], inputs['ln1_b']
    ln2_g, ln2_b = inputs['ln2_g'], inputs['ln2_b']
    scale = 1.0 / np.sqrt(C // H)

    wqf = np.empty((L, C, C), AF32)
    wkf = np.empty((L, C, C), AF32)
    wvf = np.empty((L, C, C), AF32)
    w1f = np.empty((L, C, DFF), AF32)
    bqh = np.empty((L, 128, NCH), AF32)
    bkh = np.empty((L, 128, NCH), AF32)
    bvh = np.empty((L, 128, NCH), AF32)
    boh = np.empty((L, 128, NCH), AF32)
    b1h = np.empty((L, 128, 32), AF32)
    b2h = np.empty((L, 128, NCH), AF32)
    for l in range(L):
        Wf, v = _fold(ln1_g[l], ln1_b[l], inputs['Wq'][l], inputs['bq'][l], scale)
        wqf[l], bqh[l] = Wf, _r8(v)
        Wf, v = _fold(ln1_g[l], ln1_b[l], inputs['Wk'][l], inputs['bk'][l])
        wkf[l], bkh[l] = Wf, _r8(v)
        Wf, v = _fold(ln1_g[l], ln1_b[l], inputs['Wv'][l], inputs['bv'][l])
        wvf[l], bvh[l] = Wf, _r8(v)
        boh[l] = _r8(inputs['bo'][l])
        Wf, v = _fold(ln2_g[l], ln2_b[l], inputs['W1'][l], inputs['b1'][l])
        w1f[l], b1h[l] = Wf, _r8(v)
        b2h[l] = _r8(inputs['b2'][l])
    wof = np.asarray(inputs['Wo'], AF32)
    w2f = np.asarray(inputs['W2'], AF32)

    sq = _sexp(np.abs(wqf).max())
    sk = _sexp(np.abs(wkf).max())
    sv = _sexp(np.abs(wvf).max())
    so = _sexp(np.abs(wof).max())
    s1 = _sexp(np.abs(w1f).max())
    s2 = _sexp(np.abs(w2f).max())

    wq8 = np.empty((L, NPR, 128, 2, C), AE4)
    wk8 = np.empty((L, NPR, 128, 2, C), AE4)
    wv8 = np.empty((L, NPR, 128, 2, C), AE4)
    wo8 = np.empty((L, NPR, 128, 2, C), AE4)
    w18 = np.empty((L, NPR, 4, 128, 2, 1024), AE4)
    w28 = np.empty((L, DFF // 256, 128, 2, C), AE4)
    for l in range(L):
        wq8[l] = _pack8(wqf[l], sq)
        wk8[l] = _pack8(wkf[l], sk)
        wv8[l] = _pack8(wvf[l], sv)
        wo8[l] = _pack8(wof[l], so)
        w18[l] = _pack8(w1f[l], s1).reshape(NPR, 128, 2, 4, 1024) \
            .transpose(0, 3, 1, 2, 4)
        w28[l] = _pack8(w2f[l], s2)

    # K/Q biases ride on the 2^(s+AX)-scaled K/Q tensors
    bqh *= np.float32(2.0 ** (sq + 2 * AX))
    bkh *= np.float32(2.0 ** (sk + AX))

    pwf, pv = _fold(inputs['lnf_g'], inputs['lnf_b'], inputs['proj_w'],
                    inputs['proj_b'])
    dw2f, d2v = _fold(inputs['dec_ln_g'], inputs['dec_ln_b'], inputs['dec_w2'],
                      inputs['dec_b2'])
    d1v = np.asarray(inputs['dec_b1'], AF32)

    use_bias = {
        'qkv': bool(np.abs(bqh).max() > 0 or np.abs(bkh).max() > 0),
        'v': bool(np.abs(bvh).max() > 0),
        'o': bool(np.abs(boh).max() > 0),
        'fc1': bool(np.abs(b1h).max() > 0),
        'fc2': bool(np.abs(b2h).max() > 0),
        'pw': bool(np.abs(pv).max() > 0),
        'dw1': bool(np.abs(d1v).max() > 0),
        'dw2': bool(np.abs(d2v).max() > 0),
    }
    key = (tuple(sorted(use_bias.items())), sq, sk, sv, so, s1, s2,
           tuple(sorted(MLP8)))
    if key not in _CACHE:
        _CACHE[key] = _build(use_bias, sq, sk, sv, so, s1, s2, MLP8)
    nc = _CACHE[key]
    l16 = sorted(set(range(L)) - MLP8)

    tri = np.tril(np.ones((128, 128), AF32)).T.astype(ABF16)
    ones_m = np.ones((128, 128), AF32).astype(ABF16)
    zeros_m = np.zeros((128, 128), AF32).astype(ABF16)
    shared = dict(
        emb_w=np.asarray(inputs['tok_emb_w'], AF32).astype(ABF16),
        emb_b=_r8(inputs['tok_emb_b']),
        wq=wq8, wk=wk8, wv=wv8, wo=wo8, w1=w18, w2=w28,
        bq=bqh, bk=bkh, bv=bvh, bo=boh, b1=b1h, b2=b2h,
        pw=pwf.astype(ABF16), pb=_r8(pv),
        dw1=np.asarray(inputs['dec_w1'], AF32).astype(ABF16), db1=_r8(d1v),
        dw2=dw2f.astype(ABF16), db2=np.ascontiguousarray(d2v.reshape(1, 128).T),
    )
    if l16:
        shared['w1b'] = w1f[l16].astype(ABF16)
        shared['w2b'] = w2f[l16].astype(ABF16)

    in_maps = []
    for c in range(8):
        b_, s = c // 2, c % 2
        tloc = np.concatenate([np.arange(128 * (2 * j + s), 128 * (2 * j + s) + 128)
                               for j in range(NB)])
        tnext = np.minimum(tloc + 1, T - 1)
        im = dict(shared)
        im['xloc'] = np.ascontiguousarray(x[b_][tloc].T).astype(ABF16)
        im['xshift'] = np.ascontiguousarray(x[b_][tnext].T)
        # parity-p key block j is attended by local query block jq when
        # global 2*jq+s >= 2*j+p; the diagonal jq==j needs tri when p==s,
        # everything when p<s (s=1,p=0), nothing when p>s (s=0,p=1).
        im['mask_a'] = tri if s == 0 else ones_m
        im['mask_b'] = zeros_m if s == 0 else tri
        in_maps.append(im)

    res = run_bass_kernel_spmd(nc, in_maps, list(range(8)))
    out = np.empty((B, T - 1, DIN), AF32)
    for c in range(8):
        b_, s = c // 2, c % 2
        o = res.results[c]['out'].T  # [TL, 128], DMA'd untransposed
        for j in range(NB):
            g = 2 * j + s
            t0, t1 = 128 * g, min(128 * g + 128, T - 1)
            out[b_, t0:t1] = o[128 * j:128 * j + (t1 - t0)]
    return out


# revision 28
# speedup vs baseline: 1.0345x; 1.0345x over previous
"""Dense transformer (B=4,T=1024,C=1024,H=16,L=8) on 8 TRN2 NeuronCores.

Sharding: DP over batch (4) x sequence block-cyclic (2). Core c = 2b+s handles
batch b, token blocks {j : j%2==s} (128 tokens each, 512 tokens/core).

v2 design: instead of exchanging K/V per layer, the pair cores exchange the
fp8 NORMALIZED activations x8 = LN(h)*2^AX (one chunked 2-rank AllGather of
512KB per layer) and each core recomputes K/V for the full 1024 tokens from
the gathered x8. The AllGather output is rank-ordered == parity-ordered, so
all per-core data placement is compile-time identical (SPMD); only the causal
masks differ per core and ride in as runtime inputs (mask_a for parity-0 key
blocks, mask_b for parity-1).

Q/K/V and FC1 all consume pre-normalized fp8 activations, so PSUM evictions
are single scalar-engine ops (Copy / Gelu with a constant power-of-2 scale)
instead of vector-engine tensor_tensor chains - this keeps the tensor engine
fed so the HAM clock gate stays at 8/8 (2.4GHz). Activation-table switches
(Sqrt/Exp/Gelu live in different tables, 1.28us reload) are prefetched with
dummy [128,1] ops in scalar-engine idle windows.

fp8: the weight-stationary GEMMs run fp8e4m3 MatmulPerfMode.DoubleRow
(pairs of 128-channel k-tiles per instruction). Weights scaled by per-class
powers of two; descales fold into Exp's scale, the softmax-denominator
path, the Gelu eviction scale, and scalar_tensor_tensor mult slots.
Attention (QK/AV) and the head stay bf16; residual h is f32r; all matmuls
accumulate in fp32 PSUM. MLP of layers in MLP8 runs fp8; the rest bf16.
"""
import sys
import os
import numpy as np
import ml_dtypes

sys.path.insert(0, '/opt/trn_rl_repo')

import concourse.bass as bass  # noqa: F401
import concourse.tile as tile
from concourse import bacc, mybir
from concourse.bass_utils import run_bass_kernel_spmd

F = mybir.ActivationFunctionType
ALU = mybir.AluOpType
PM = mybir.MatmulPerfMode
dt = mybir.dt
AF32 = np.float32
ABF16 = ml_dtypes.bfloat16
AE4 = ml_dtypes.float8_e4m3fn

B, T, C, H, L = 4, 1024, 1024, 16, 8
DIN, DOUT, DH = 128, 256, 64
TL = 512
NB = 4
NCH = C // 128
NPR = NCH // 2
DFF = 4 * C
EPS = 1e-5

_CACHE = {}
L_RUN = int(os.environ.get('KLAYERS', str(L)))

# activation pre-scale exponents (pow2) applied before fp8 casts to keep
# small values out of e4m3 subnormal range; descale folds into existing
# eviction constants.
AX = 3   # normalized x8 (QKV stationary input)
AY = 5   # attention output y8

# layers whose MLP runs fp8 DoubleRow; the rest run bf16. fp8 MLP noise
# dominates the error budget, and EARLY layer noise is ~4x more damaging
# than late (it compounds through the stack) - so late layers go fp8.
MLP8 = frozenset(
    int(x) for x in os.environ.get('KMLP8', '3,4,5,6,7').split(',') if x != '')


def _build(use_bias, sq, sk, sv, so, s1, s2, mlp8):
    """use_bias: dict of bools; s*: power-of-2 exponents of the fp8 weight
    scaling per matrix class (weights stored as W*2^s); mlp8: layer indices
    whose MLP runs fp8 DoubleRow (others run bf16 for accuracy)."""
    nc = bacc.Bacc("TRN2", target_bir_lowering=False, debug=False, num_devices=8)
    l16 = sorted(set(range(L)) - set(mlp8))
    l16idx = {li: i for i, li in enumerate(l16)}

    def par(name, shape, dtp):
        return nc.declare_dram_parameter(name, list(shape), dtp, isOutput=False)

    xloc = par('xloc', [128, TL], dt.bfloat16)
    xshift = par('xshift', [128, TL], dt.float32)
    emb_w = par('emb_w', [128, C], dt.bfloat16)
    emb_b = par('emb_b', [128, NCH], dt.float32)
    wq = par('wq', [L, NPR, 128, 2, C], dt.float8e4)
    wk = par('wk', [L, NPR, 128, 2, C], dt.float8e4)
    wv = par('wv', [L, NPR, 128, 2, C], dt.float8e4)
    bq = par('bq', [L, 128, NCH], dt.float32)
    bk = par('bk', [L, 128, NCH], dt.float32)
    bv = par('bv', [L, 128, NCH], dt.float32)
    wo = par('wo', [L, NPR, 128, 2, C], dt.float8e4)
    bo = par('bo', [L, 128, NCH], dt.float32)
    w1 = par('w1', [L, NPR, 4, 128, 2, 1024], dt.float8e4)
    b1 = par('b1', [L, 128, 32], dt.float32)
    w2 = par('w2', [L, DFF // 256, 128, 2, C], dt.float8e4)
    b2 = par('b2', [L, 128, NCH], dt.float32)
    if l16:
        w1b = par('w1b', [len(l16), C, DFF], dt.bfloat16)
        w2b = par('w2b', [len(l16), DFF, C], dt.bfloat16)
    pw = par('pw', [C, DOUT], dt.bfloat16)
    pb = par('pb', [128, 2], dt.float32)
    dw1 = par('dw1', [DOUT, DOUT], dt.bfloat16)
    db1 = par('db1', [128, 2], dt.float32)
    dw2 = par('dw2', [DOUT, DIN], dt.bfloat16)
    db2 = par('db2', [128, 1], dt.float32)
    mask_a = par('mask_a', [128, 128], dt.bfloat16)
    mask_b = par('mask_b', [128, 128], dt.bfloat16)
    out_p = nc.declare_dram_parameter('out', [128, TL], dt.float32, isOutput=True)

    exp_scale = float(2.0 ** (-(sq + sk + 4 * AX)))

    with tile.TileContext(nc, num_cores=8) as tc:
        with tc.tile_pool(name='persist', bufs=1) as pp, \
             tc.tile_pool(name='sbwork', bufs=1) as wkp, \
             tc.tile_pool(name='wslab', bufs=1) as wsp, \
             tc.tile_pool(name='small', bufs=1) as smp, \
             tc.tile_pool(name='dram', bufs=2, space='DRAM') as drp:

            hT = pp.tile([128, NCH, TL], dt.float32r, name='hT')
            h8 = pp.tile([128, NCH, TL], dt.float8e4, name='h8')
            QT = pp.tile([128, NCH, TL], dt.bfloat16, name='QT')
            # K for full T, parity-indexed: [:, c, par, t]
            KT = pp.tile([128, NCH, 2, TL], dt.bfloat16, name='KT')
            # V stored head-group-major [g, par, j, hh, 128]: cols 0:64 are V
            # data, cols 64:128 are ones so the AV matmul emits the softmax
            # denominator replicated on PSUM partitions 64:128.
            Vv = pp.tile([128, 4, 2, NB, 4, 128], dt.bfloat16, name='Vv')
            y8 = pp.tile([128, NCH, TL], dt.float8e4, name='y8')
            mska = pp.tile([128, 128], dt.bfloat16, name='mska')
            mskb = pp.tile([128, 128], dt.bfloat16, name='mskb')
            ones_mf = pp.tile([128, 128], dt.float32, name='ones_mf')
            ones_mat = pp.tile([128, 128], dt.float32r, name='ones_mat')
            ones_f32 = pp.tile([128, 1], dt.float32, name='ones_f32')
            eps_t = pp.tile([128, 1], dt.float32, name='eps_t')
            dscO = pp.tile([128, 1], dt.float32, name='dscO')
            dscW2 = pp.tile([128, 1], dt.float32, name='dscW2')
            cmAX = pp.tile([128, 1], dt.float32, name='cmAX')
            sbT = pp.tile([128, 2, NB], dt.float32, name='sbT')
            ones_h16 = pp.tile([128, 1], dt.float16, name='ones_h16')
            dum = pp.tile([128, 1], dt.float32, name='dum')
            xl_sb = pp.tile([128, TL], dt.bfloat16, name='xl_sb')
            xsh_sb = pp.tile([128, TL], dt.float32, name='xsh_sb')
            featsT = pp.tile([128, 2, TL], dt.bfloat16, name='featsT')
            zT = pp.tile([128, 2, TL], dt.float32r, name='zT')
            out_sb = pp.tile([128, TL], dt.float32, name='out_sb')

            nc.sync.dma_start(mska[:], mask_a[:])
            nc.sync.dma_start(mskb[:], mask_b[:])
            nc.sync.dma_start(xl_sb[:], xloc[:])
            nc.sync.dma_start(xsh_sb[:], xshift[:])
            nc.vector.memset(ones_f32[:], 1.0)
            nc.vector.memset(ones_mf[:], 1.0)
            nc.vector.tensor_copy(ones_mat[:], ones_mf[:])
            nc.vector.memset(eps_t[:], EPS)
            nc.vector.memset(dscO[:], float(2.0 ** (-(so + AY))))
            nc.vector.memset(dscW2[:], float(2.0 ** (-s2)))
            nc.vector.memset(cmAX[:], float(2.0 ** (-AX)))
            nc.vector.memset(ones_h16[:], 1.0)
            nc.vector.memset(Vv[:, :, :, :, :, 64:128], 1.0)

            psA = None

            def ln_scale(src_fn, nch, sexp=0):
                """[128, TL] f32 of rstd*2^sexp for channel slices
                src_fn(c) -> [128, TL]. Stats are computed replicated on all
                128 partitions (ones matrix as matmul lhsT). Short tail:
                mu^2 comes from Square straight off the sum PSUM, eps rides
                the var copy, 2^(2*sexp) rides the Sqrt scale, and the
                reciprocal is the 1-op approx (51 ULP - far below the fp8
                noise this feeds)."""
                ps_su = psA.tile([128, TL], dt.float32, tag='ps', bufs=8,
                                 name='ps_su')
                for c in range(nch):
                    nc.tensor.matmul(ps_su[:], ones_mat[:], src_fn(c),
                                     start=(c == 0), stop=(c == nch - 1))
                ps_sq = psA.tile([128, TL], dt.float32, tag='ps', bufs=8,
                                 name='ps_sq')
                for c in range(nch):
                    sq_t = wkp.tile([128, TL], dt.float32r, tag='sq', bufs=2,
                                    name='sq')
                    nc.scalar.activation(sq_t[:], src_fn(c), F.Square)
                    nc.tensor.matmul(ps_sq[:], ones_mat[:], sq_t[:],
                                     start=(c == 0), stop=(c == nch - 1))
                scr = smp.tile([128, TL], dt.float32, tag='lnscr', bufs=1,
                               name='scr')
                nc.scalar.activation(scr[:], ps_su[:], F.Square,
                                     scale=1.0 / (nch * 128))
                var = smp.tile([128, TL], dt.float32, tag='lnvar', bufs=1,
                               name='var')
                nc.scalar.activation(var[:], ps_sq[:], F.Copy,
                                     scale=1.0 / (nch * 128), bias=EPS)
                nc.vector.tensor_sub(var[:], var[:], scr[:])
                sd = smp.tile([128, TL], dt.float32, tag='lnsd', bufs=1,
                              name='sd')
                nc.scalar.activation(sd[:], var[:], F.Sqrt,
                                     scale=float(2.0 ** (-2 * sexp)))
                sb = wkp.tile([128, TL], dt.float32, tag='lnsb', bufs=3,
                              name='sb')
                nc.vector.reciprocal_approx_fast(sb[:], sd[:])
                return sb

            def prefetch(fn):
                # dummy [128,1] activation: loads fn's table while the
                # scalar engine is otherwise idle, off the critical path
                nc.scalar.activation(dum[:], eps_t[:], fn)

            def matphase8(src_sel, w_ap, l, npr_in, nch_out, out_cb, halves=1):
                """fp8 DoubleRow: out[co] = sum_p pair(w).T @ pair(src).
                src_sel(p) gives the [128, 2, TL] fp8 rhs for k-pair p.
                halves=2 splits the output channels into two 4-bank PSUM
                groups so the phase starts while the previous phase's other
                banks still drain."""
                nh = nch_out // halves
                for hh in range(halves):
                    pss = [psA.tile([128, TL], dt.float32, tag='ps', bufs=8,
                                    name=f'pp{co}') for co in range(nh)]
                    for p in range(npr_in):
                        slab = wsp.tile([128, 2, nh * 128], dt.float8e4,
                                        tag='wslab8', bufs=8, name='slab')
                        wsl = w_ap[l, p] if l is not None else w_ap[p]
                        nc.sync.dma_start(
                            slab[:], wsl[:, :, hh * nh * 128:(hh + 1) * nh * 128])
                        for co in range(nh):
                            nc.tensor.matmul(pss[co][:],
                                             slab[:, :, co * 128:(co + 1) * 128],
                                             src_sel(p),
                                             start=(p == 0),
                                             stop=(p == npr_in - 1),
                                             perf_mode=PM.DoubleRow)
                    for co in range(nh):
                        out_cb(hh * nh + co, pss[co])

            def matphase(src, w_ap, l, nch_in, nch_out, out_cb, wtag, wdt):
                """bf16: out[co] = sum_ci w[ci,co].T @ src[:,ci,:]."""
                pss = [psA.tile([128, TL], dt.float32, tag='ps', bufs=8,
                                name=f'pp{co}') for co in range(nch_out)]
                for ci in range(nch_in):
                    slab = wsp.tile([128, nch_out * 128], wdt, tag=wtag,
                                    bufs=(4 if wtag == 'wslab' else 5), name='slab')
                    src_w = w_ap[l, ci * 128:(ci + 1) * 128, :] if l is not None \
                        else w_ap[ci * 128:(ci + 1) * 128, :]
                    nc.sync.dma_start(slab[:], src_w)
                    for co in range(nch_out):
                        nc.tensor.matmul(pss[co][:], slab[:, co * 128:(co + 1) * 128],
                                         src[:, ci, :], start=(ci == 0),
                                         stop=(ci == nch_in - 1))
                for co in range(nch_out):
                    out_cb(co, pss[co])

            # ---------------- embed ----------------
            with tc.tile_pool(name='psE', bufs=1, space='PSUM') as psA:
                embs = wsp.tile([128, C], dt.bfloat16, tag='wslabb', bufs=5,
                                name='embs')
                nc.sync.dma_start(embs[:], emb_w[:])
                ebias = smp.tile([128, NCH], dt.float32, tag='bias8', bufs=2,
                                 name='ebias')
                nc.sync.dma_start(ebias[:], emb_b[:])
                for co in range(NCH):
                    ps = psA.tile([128, TL], dt.float32, tag='ps', bufs=8,
                                  name=f'pe{co}')
                    nc.tensor.matmul(ps[:], embs[:, co * 128:(co + 1) * 128],
                                     xl_sb[:], start=True, stop=True)
                    nc.scalar.activation(hT[:, co, :], ps[:], F.Identity,
                                         bias=ebias[:, co:co + 1])

            # ---------------- layers ----------------
            for li in range(L_RUN):
                with tc.tile_pool(name=f'psA{li}', bufs=1, space='PSUM') as psA:
                    # h8f shares one ring slot with the MLP input (m8/m16):
                    # their lifetimes are disjoint within a layer.
                    h8f = wkp.tile([128, NCH, 2, TL], dt.float8e4, tag='xph',
                                   bufs=1, name='h8f')
                    # h8 = h*2^AX needs no LN stats, so its AllGather
                    # launches the moment FC2 of the previous layer lands -
                    # LN happens AFTER the exchange, on the gathered h8f, so
                    # no rstd round-trip sits on the collective's critical
                    # path. Per-token rstd factors out of every projection
                    # matmul and rides on the evictions.
                    outbs = []
                    for ch in range(NPR):
                        for cc in (2 * ch, 2 * ch + 1):
                            nc.vector.tensor_scalar_mul(
                                h8[:, cc, :], hT[:, cc, :], float(2.0 ** AX))
                        inb = drp.tile([128, 2 * TL], dt.float8e4,
                                       tag='inbX', bufs=8, name='inbX')
                        outb = drp.tile([256, 2 * TL], dt.float8e4,
                                        tag='outbX', bufs=8, name='outbX')
                        nc.sync.dma_start(
                            inb[:], h8[:, 2 * ch:2 * ch + 2, :])
                        nc.gpsimd.collective_compute(
                            "AllGather", ALU.bypass,
                            replica_groups=[[0, 1], [2, 3], [4, 5], [6, 7]],
                            ins=[inb.opt()], outs=[outb.opt()])
                        outbs.append(outb)
                        # gathered chunks land parity-indexed (AllGather
                        # output is rank-ordered and rank pair index ==
                        # token parity): identical placement on every core.
                        for par in range(2):
                            nc.sync.dma_start(
                                h8f[:, 2 * ch:2 * ch + 2, par, :],
                                outb[128 * par:128 * par + 128, :])

                    sb1 = ln_scale(lambda c: hT[:, c, :], NCH, sexp=AX)

                    qb = smp.tile([128, NCH], dt.float32, tag='bias8', bufs=2,
                                  name='qb')
                    if use_bias['qkv']:
                        nc.sync.dma_start(qb[:], bq[li])

                    def evict_q(co, ps):
                        nc.vector.tensor_mul(QT[:, co, :], ps[:], sb1[:])
                        if use_bias['qkv']:
                            nc.vector.tensor_scalar_add(
                                QT[:, co, :], QT[:, co, :], qb[:, co:co + 1])
                    matphase8(lambda p: h8[:, 2 * p:2 * p + 2, :],
                              wq, li, NPR, NCH, evict_q, halves=2)

                    # The per-token rstd row is replicated across partitions,
                    # so a tiny AllGather of sb1 (fp16, 128KB) hands every
                    # core BOTH parities' rstd already in replicated layout -
                    # no LN pass over the gathered h8 needed, and the peer
                    # rstd is the peer's exact f32-h statistics.
                    sb16 = wkp.tile([128, TL], dt.float16, tag='sb16',
                                    bufs=1, name='sb16')
                    nc.scalar.activation(sb16[:], sb1[:], F.Copy)
                    inbS = drp.tile([128, TL], dt.float16, tag='inbS',
                                    bufs=2, name='inbS')
                    outbS = drp.tile([256, TL], dt.float16, tag='outbS',
                                     bufs=2, name='outbS')
                    nc.sync.dma_start(inbS[:], sb16[:])
                    nc.gpsimd.collective_compute(
                        "AllGather", ALU.bypass,
                        replica_groups=[[0, 1], [2, 3], [4, 5], [6, 7]],
                        ins=[inbS.opt()], outs=[outbS.opt()])
                    sbp = []
                    for par in range(2):
                        sbpt = wkp.tile([128, TL], dt.float16, tag='sbp',
                                        bufs=2, name='sbp')
                        nc.sync.dma_start(sbpt[:],
                                          outbS[128 * par:128 * par + 128, :])
                        sbp.append(sbpt)
                    # transposed rstd columns for the V eviction scale:
                    # sbT[:, par, tb] = rstd of token block tb (parity par),
                    # one token per partition. A contract-1 matmul moves the
                    # replicated row onto partitions.
                    for par in range(2):
                        for tb in range(NB):
                            psT = psA.tile([128, TL], dt.float32, tag='ps',
                                           bufs=8, name='psT')
                            nc.tensor.matmul(
                                psT[:, 0:1],
                                sbp[par][0:1, tb * 128:(tb + 1) * 128],
                                ones_h16[0:1, 0:1], start=True, stop=True)
                            nc.scalar.activation(sbT[:, par, tb:tb + 1],
                                                 psT[:, 0:1], F.Copy)

                    kb = smp.tile([128, NCH], dt.float32, tag='bias8', bufs=2,
                                  name='kb')
                    if use_bias['qkv']:
                        nc.sync.dma_start(kb[:], bk[li])

                    for kpar in range(2):
                        def evict_k(co, ps, kpar=kpar):
                            nc.vector.tensor_mul(KT[:, co, kpar, :], ps[:],
                                                 sbp[kpar][:])
                            if use_bias['qkv']:
                                nc.vector.tensor_scalar_add(
                                    KT[:, co, kpar, :], KT[:, co, kpar, :],
                                    kb[:, co:co + 1])
                        matphase8(lambda p, kpar=kpar:
                                  h8f[:, 2 * p:2 * p + 2, kpar, :],
                                  wk, li, NPR, NCH, evict_k, halves=2)

                    # V proj (token-major): psv[tb] = h8f[:,:,par,tb].T @ Wv
                    # half; rstd applies at evict via the transposed column.
                    # Weight slabs hoisted and reused across the 4 (par, dvh)
                    # quarter-phases; 4 PSUM banks per phase.
                    vslabs = []
                    for p in range(NPR):
                        vs = wsp.tile([128, 2, C], dt.float8e4, tag='vslab',
                                      bufs=4, name='vslab')
                        nc.sync.dma_start(vs[:], wv[li, p])
                        vslabs.append(vs)
                    for vpar in range(2):
                        for dvh in range(2):
                            psv = [psA.tile([128, TL], dt.float32, tag='ps',
                                            bufs=8, name=f'pv{i}')
                                   for i in range(NB)]
                            for p in range(NPR):
                                for tb in range(NB):
                                    nc.tensor.matmul(
                                        psv[tb][:],
                                        h8f[:, 2 * p:2 * p + 2, vpar,
                                            tb * 128:(tb + 1) * 128],
                                        vslabs[p][:, :, dvh * 512:(dvh + 1) * 512],
                                        start=(p == 0), stop=(p == NPR - 1),
                                        perf_mode=PM.DoubleRow)
                            for tb in range(NB):
                                src = psv[tb][:].rearrange(
                                    "p (h e) -> p h e", e=64)
                                for gg in range(2):
                                    dst = Vv[:, 2 * dvh + gg, vpar, tb, 0:4, 0:64]
                                    nc.scalar.activation(
                                        dst, src[:, 4 * gg:4 * gg + 4, :],
                                        F.Copy,
                                        scale=sbT[:, vpar, tb:tb + 1])

                with tc.tile_pool(name=f'psB{li}', bufs=1, space='PSUM') as psB:
                    vbl = smp.tile([128, NCH], dt.float32, tag='bias8v', bufs=2,
                                   name='vbl')
                    if use_bias['v']:
                        nc.sync.dma_start(vbl[:], bv[li])

                    def att_pass(cp, psy, apar):
                        msk = mska if apar == 0 else mskb
                        for j in range(NB):
                            qs = 128 * j
                            qn = TL - qs
                            pssc = psB.tile([128, 2, TL], dt.float32, tag='pssc',
                                            bufs=2, name='pssc')
                            for hp in range(2):
                                nc.tensor.matmul(
                                    pssc[:, hp, 0:qn],
                                    KT[hp * 64:(hp + 1) * 64, cp, apar,
                                       j * 128:(j + 1) * 128],
                                    QT[hp * 64:(hp + 1) * 64, cp, qs:TL],
                                    start=True, stop=True)
                            et = wkp.tile([128, 2, qn], dt.bfloat16, tag='et',
                                          bufs=3, name='et')
                            nc.scalar.activation(et[:], pssc[:, :, 0:qn], F.Exp,
                                                 scale=exp_scale)
                            for hp in range(2):
                                nc.vector.tensor_mul(et[:, hp, 0:128],
                                                     et[:, hp, 0:128], msk[:])
                            for hp in range(2):
                                nc.tensor.matmul(
                                    psy[:, hp, qs:TL],
                                    Vv[:, cp // 2, apar, j, 2 * (cp % 2) + hp, :],
                                    et[:, hp, :],
                                    start=(apar == 0 and j == 0),
                                    stop=(apar == 1 and j == NB - 1))

                    def att_evict(cp, psy):
                        # fold V's 2^(sv+AX) and y8's 2^AY prescale into the
                        # numerator copy; psy[64:128] is the softmax
                        # denominator replicated across 64 partitions.
                        ysb = wkp.tile([64, 2, TL], dt.bfloat16, tag='ysb', bufs=1,
                                       name='ysb')
                        nc.scalar.activation(ysb[:], psy[0:64, :, :], F.Copy,
                                             scale=float(2.0 ** (AY - sv - 2 * AX)))
                        rbd = wkp.tile([64, 2, TL], dt.float32, tag='rbd',
                                       bufs=1, name='rbd')
                        nc.scalar.activation(rbd[:], psy[64:128, :, :], F.Copy)
                        rb = wkp.tile([64, 2, TL], dt.float32, tag='rb', bufs=1,
                                      name='rb')
                        nc.vector.reciprocal_approx_fast(rb[:], rbd[:])
                        for hp in range(2):
                            nc.vector.tensor_mul(y8[hp * 64:(hp + 1) * 64, cp, :],
                                                 ysb[:, hp, :], rb[:, hp, :])
                            if use_bias['v']:
                                nc.vector.tensor_scalar_add(
                                    y8[hp * 64:(hp + 1) * 64, cp, :],
                                    y8[hp * 64:(hp + 1) * 64, cp, :],
                                    vbl[hp * 64:(hp + 1) * 64, cp:cp + 1])

                    for cw in range(4):
                        for ci in range(2):
                            cp = 2 * cw + ci
                            psy = psB.tile([128, 2, TL], dt.float32, tag='psy',
                                           bufs=2, name='psy')
                            att_pass(cp, psy, 0)
                            att_pass(cp, psy, 1)
                            att_evict(cp, psy)

                with tc.tile_pool(name=f'psC{li}', bufs=1, space='PSUM') as psA:
                    obias = smp.tile([128, NCH], dt.float32, tag='bias8o',
                                     bufs=2, name='obias')
                    if use_bias['o']:
                        nc.sync.dma_start(obias[:], bo[li])

                    def evict_proj(co, ps):
                        nc.vector.scalar_tensor_tensor(
                            hT[:, co, :], ps[:], dscO[:, 0:1], hT[:, co, :],
                            ALU.mult, ALU.add)
                        if use_bias['o']:
                            nc.vector.tensor_scalar_add(
                                hT[:, co, :], hT[:, co, :], obias[:, co:co + 1])
                    matphase8(lambda p: y8[:, 2 * p:2 * p + 2, :],
                              wo, li, NPR, NCH, evict_proj, halves=2)

                    b1s = smp.tile([128, 32], dt.float32, tag='b1s', bufs=2,
                                   name='b1s')
                    if use_bias['fc1']:
                        nc.sync.dma_start(b1s[:], b1[li])
                    b2s = smp.tile([128, NCH], dt.float32, tag='bias8', bufs=2,
                                   name='b2s')
                    if use_bias['fc2']:
                        nc.sync.dma_start(b2s[:], b2[li])

                    sb2 = ln_scale(lambda c: hT[:, c, :], NCH, sexp=AX)
                    prefetch(F.Gelu)

                    if li in mlp8:
                        g1sc = float(2.0 ** (-(s1 + AX)))
                        m8 = wkp.tile([128, NCH, TL], dt.float8e4, tag='xph',
                                      bufs=1, name='m8')
                        for cc in range(NCH):
                            nc.vector.tensor_mul(m8[:, cc, :], hT[:, cc, :],
                                                 sb2[:])
                        m_act = wkp.tile([128, 32, TL], dt.float8e4, tag='mact',
                                         bufs=1, name='m_act8')
                        for fog in range(4):
                            psf = [psA.tile([128, TL], dt.float32, tag='ps',
                                            bufs=8, name=f'pf{i}')
                                   for i in range(8)]
                            for p in range(NPR):
                                slab = wsp.tile([128, 2, 1024], dt.float8e4,
                                                tag='wslab8', bufs=8,
                                                name='f1slab')
                                nc.sync.dma_start(slab[:], w1[li, p, fog])
                                for fo in range(8):
                                    nc.tensor.matmul(
                                        psf[fo][:],
                                        slab[:, :, fo * 128:(fo + 1) * 128],
                                        m8[:, 2 * p:2 * p + 2, :],
                                        start=(p == 0), stop=(p == NPR - 1),
                                        perf_mode=PM.DoubleRow)
                            for fo in range(8):
                                fi = fog * 8 + fo
                                nc.scalar.activation(
                                    m_act[:, fi, :], psf[fo][:], F.Gelu,
                                    bias=(b1s[:, fi:fi + 1]
                                          if use_bias['fc1'] else 0.0),
                                    scale=g1sc)
                        for coh in range(2):
                            psm = [psA.tile([128, TL], dt.float32, tag='ps',
                                            bufs=8, name=f'pm{i}')
                                   for i in range(4)]
                            for p in range(16):
                                slab = wsp.tile([128, 2, C // 2], dt.float8e4,
                                                tag='wslab8', bufs=8,
                                                name='f2slab')
                                nc.sync.dma_start(
                                    slab[:],
                                    w2[li, p, :, :, coh * 512:(coh + 1) * 512])
                                for c4 in range(4):
                                    nc.tensor.matmul(
                                        psm[c4][:],
                                        slab[:, :, c4 * 128:(c4 + 1) * 128],
                                        m_act[:, 2 * p:2 * p + 2, :],
                                        start=(p == 0), stop=(p == 15),
                                        perf_mode=PM.DoubleRow)
                            for c4 in range(4):
                                co = coh * 4 + c4
                                nc.vector.scalar_tensor_tensor(
                                    hT[:, co, :], psm[c4][:], dscW2[:, 0:1],
                                    hT[:, co, :], ALU.mult, ALU.add)
                                if use_bias['fc2']:
                                    nc.vector.tensor_scalar_add(
                                        hT[:, co, :], hT[:, co, :],
                                        b2s[:, co:co + 1])
                    else:
                        # bf16 MLP layer (precision recovery): normalized
                        # input in bf16, gelu straight off PSUM.
                        m16 = wkp.tile([128, NCH, TL], dt.bfloat16, tag='xph',
                                       bufs=1, name='m16')
                        for cc in range(NCH):
                            nc.vector.scalar_tensor_tensor(
                                m16[:, cc, :], hT[:, cc, :], cmAX[:, 0:1],
                                sb2[:], ALU.mult, ALU.mult)
                        m16a = wkp.tile([128, 32, TL], dt.bfloat16, tag='mact',
                                        bufs=1, name='m16a')
                        lb = l16idx[li]
                        for fog in range(4):
                            psf = [psA.tile([128, TL], dt.float32, tag='ps',
                                            bufs=8, name=f'pf{i}')
                                   for i in range(8)]
                            for ci in range(NCH):
                                slab = wsp.tile([128, C], dt.bfloat16,
                                                tag='wslabb', bufs=5,
                                                name='f1slabb')
                                nc.sync.dma_start(
                                    slab[:], w1b[lb, ci * 128:(ci + 1) * 128,
                                                 fog * 1024:(fog + 1) * 1024])
                                for fo in range(8):
                                    nc.tensor.matmul(
                                        psf[fo][:],
                                        slab[:, fo * 128:(fo + 1) * 128],
                                        m16[:, ci, :], start=(ci == 0),
                                        stop=(ci == NCH - 1))
                            for fo in range(8):
                                fi = fog * 8 + fo
                                nc.scalar.activation(
                                    m16a[:, fi, :], psf[fo][:], F.Gelu,
                                    bias=(b1s[:, fi:fi + 1]
                                          if use_bias['fc1'] else 0.0))
                        for coh in range(2):
                            psm = [psA.tile([128, TL], dt.float32, tag='ps',
                                            bufs=8, name=f'pm{i}')
                                   for i in range(4)]
                            for fi in range(32):
                                slab = wsp.tile([128, C // 2], dt.bfloat16,
                                                tag='wslabb', bufs=5,
                                                name='f2slabb')
                                nc.sync.dma_start(
                                    slab[:], w2b[lb, fi * 128:(fi + 1) * 128,
                                                 coh * 512:(coh + 1) * 512])
                                for c4 in range(4):
                                    nc.tensor.matmul(
                                        psm[c4][:],
                                        slab[:, c4 * 128:(c4 + 1) * 128],
                                        m16a[:, fi, :], start=(fi == 0),
                                        stop=(fi == 31))
                            for c4 in range(4):
                                co = coh * 4 + c4
                                nc.vector.scalar_tensor_tensor(
                                    hT[:, co, :], psm[c4][:], ones_f32[:, 0:1],
                                    hT[:, co, :], ALU.mult, ALU.add)
                                if use_bias['fc2']:
                                    nc.vector.tensor_scalar_add(
                                        hT[:, co, :], hT[:, co, :],
                                        b2s[:, co:co + 1])

            # ---------------- head ----------------
            with tc.tile_pool(name='psH', bufs=1, space='PSUM') as psA:
                hTb = wkp.tile([128, NCH, TL], dt.bfloat16, tag='mact', bufs=1,
                               name='hTb')
                for co in range(NCH):
                    nc.vector.tensor_copy(hTb[:, co, :], hT[:, co, :])
                sbf = ln_scale(lambda c: hT[:, c, :], NCH)
                pbias = smp.tile([128, 2], dt.float32, tag='bias2', bufs=2,
                                 name='pbias')
                if use_bias['pw']:
                    nc.sync.dma_start(pbias[:], pb[:])

                def evict_pw(co, ps):
                    nc.vector.tensor_mul(featsT[:, co, :], ps[:], sbf[:])
                    if use_bias['pw']:
                        nc.vector.tensor_scalar_add(featsT[:, co, :],
                                                    featsT[:, co, :],
                                                    pbias[:, co:co + 1])
                matphase(hTb, pw, None, NCH, 2, evict_pw, 'wslabb', dt.bfloat16)

                d1b = smp.tile([128, 2], dt.float32, tag='bias2', bufs=2, name='d1b')
                if use_bias['dw1']:
                    nc.sync.dma_start(d1b[:], db1[:])

                def evict_d1(co, ps):
                    nc.scalar.activation(zT[:, co, :], ps[:], F.Tanh,
                                         bias=(d1b[:, co:co + 1]
                                               if use_bias['dw1'] else 0.0))
                matphase(featsT, dw1, None, 2, 2, evict_d1, 'wslabb', dt.bfloat16)

                sbz = ln_scale(lambda c: zT[:, c, :], 2)
                zb = wkp.tile([128, 2, TL], dt.bfloat16, tag='zb', bufs=1, name='zb')
                for co in range(2):
                    nc.vector.tensor_copy(zb[:, co, :], zT[:, co, :])
                d2b = smp.tile([128, 1], dt.float32, tag='bias2', bufs=2, name='d2b')
                if use_bias['dw2']:
                    nc.sync.dma_start(d2b[:], db2[:])

                def evict_out(co, ps):
                    ptmp = wkp.tile([128, TL], dt.float32, tag='ptmp', bufs=1,
                                    name='ptmp')
                    nc.vector.tensor_mul(ptmp[:], ps[:], sbz[:])
                    if use_bias['dw2']:
                        nc.vector.scalar_tensor_tensor(out_sb[:], ptmp[:],
                                                       d2b[:, 0:1], xsh_sb[:],
                                                       ALU.add, ALU.subtract)
                    else:
                        nc.vector.tensor_sub(out_sb[:], ptmp[:], xsh_sb[:])
                matphase(zb, dw2, None, 2, 1, evict_out, 'wslabb', dt.bfloat16)
                nc.sync.dma_start(out_p[:], out_sb[:])

    nc.compile()
    return nc


def _fold(g, b, W, bias, scl=1.0):
    """LN(x;g,b) @ W + bias == (x @ W'')*rstd + v with the mean folded in."""
    g = np.asarray(g, np.float64)
    W = np.asarray(W, np.float64)
    u = g @ W
    Wf = (g[:, None] * W - u[None, :] / W.shape[0]) * scl
    v = (np.asarray(b, np.float64) @ W + np.asarray(bias, np.float64)) * scl
    return Wf.astype(AF32), v.astype(AF32)


def _r8(v):
    return np.ascontiguousarray(np.asarray(v, AF32).reshape(-1, 128).T)


def _sexp(absmax):
    """Largest s with absmax * 2^s <= 240."""
    return int(np.floor(np.log2(240.0 / max(absmax, 1e-30))))


def _pack8(W, s):
    """[Cin, Cout] f32 -> [Cin//256, 128, 2, Cout] fp8e4 scaled by 2^s."""
    cin, cout = W.shape
    Wr = (np.asarray(W, AF32) * np.float32(2.0 ** s)).reshape(
        cin // 256, 2, 128, cout).transpose(0, 2, 1, 3)
    return np.ascontiguousarray(Wr).astype(AE4)


def kernel(**inputs):
    x = np.asarray(inputs['tokens'], AF32).reshape(B, T, DIN)
    ln1_g, ln1_b = inputs['ln1_g'], inputs['ln1_b']
    ln2_g, ln2_b = inputs['ln2_g'], inputs['ln2_b']
    scale = 1.0 / np.sqrt(C // H)

    wqf = np.empty((L, C, C), AF32)
    wkf = np.empty((L, C, C), AF32)
    wvf = np.empty((L, C, C), AF32)
    w1f = np.empty((L, C, DFF), AF32)
    bqh = np.empty((L, 128, NCH), AF32)
    bkh = np.empty((L, 128, NCH), AF32)
    bvh = np.empty((L, 128, NCH), AF32)
    boh = np.empty((L, 128, NCH), AF32)
    b1h = np.empty((L, 128, 32), AF32)
    b2h = np.empty((L, 128, NCH), AF32)
    for l in range(L):
        Wf, v = _fold(ln1_g[l], ln1_b[l], inputs['Wq'][l], inputs['bq'][l], scale)
        wqf[l], bqh[l] = Wf, _r8(v)
        Wf, v = _fold(ln1_g[l], ln1_b[l], inputs['Wk'][l], inputs['bk'][l])
        wkf[l], bkh[l] = Wf, _r8(v)
        Wf, v = _fold(ln1_g[l], ln1_b[l], inputs['Wv'][l], inputs['bv'][l])
        wvf[l], bvh[l] = Wf, _r8(v)
        boh[l] = _r8(inputs['bo'][l])
        Wf, v = _fold(ln2_g[l], ln2_b[l], inputs['W1'][l], inputs['b1'][l])
        w1f[l], b1h[l] = Wf, _r8(v)
        b2h[l] = _r8(inputs['b2'][l])
    wof = np.asarray(inputs['Wo'], AF32)
    w2f = np.asarray(inputs['W2'], AF32)

    sq = _sexp(np.abs(wqf).max())
    sk = _sexp(np.abs(wkf).max())
    sv = _sexp(np.abs(wvf).max())
    so = _sexp(np.abs(wof).max())
    s1 = _sexp(np.abs(w1f).max())
    s2 = _sexp(np.abs(w2f).max())

    wq8 = np.empty((L, NPR, 128, 2, C), AE4)
    wk8 = np.empty((L, NPR, 128, 2, C), AE4)
    wv8 = np.empty((L, NPR, 128, 2, C), AE4)
    wo8 = np.empty((L, NPR, 128, 2, C), AE4)
    w18 = np.empty((L, NPR, 4, 128, 2, 1024), AE4)
    w28 = np.empty((L, DFF // 256, 128, 2, C), AE4)
    for l in range(L):
        wq8[l] = _pack8(wqf[l], sq)
        wk8[l] = _pack8(wkf[l], sk)
        wv8[l] = _pack8(wvf[l], sv)
        wo8[l] = _pack8(wof[l], so)
        w18[l] = _pack8(w1f[l], s1).reshape(NPR, 128, 2, 4, 1024) \
            .transpose(0, 3, 1, 2, 4)
        w28[l] = _pack8(w2f[l], s2)

    # K/Q biases ride on the 2^(s+AX)-scaled K/Q tensors
    bqh *= np.float32(2.0 ** (sq + 2 * AX))
    bkh *= np.float32(2.0 ** (sk + 2 * AX))

    pwf, pv = _fold(inputs['lnf_g'], inputs['lnf_b'], inputs['proj_w'],
                    inputs['proj_b'])
    dw2f, d2v = _fold(inputs['dec_ln_g'], inputs['dec_ln_b'], inputs['dec_w2'],
                      inputs['dec_b2'])
    d1v = np.asarray(inputs['dec_b1'], AF32)

    use_bias = {
        'qkv': bool(np.abs(bqh).max() > 0 or np.abs(bkh).max() > 0),
        'v': bool(np.abs(bvh).max() > 0),
        'o': bool(np.abs(boh).max() > 0),
        'fc1': bool(np.abs(b1h).max() > 0),
        'fc2': bool(np.abs(b2h).max() > 0),
        'pw': bool(np.abs(pv).max() > 0),
        'dw1': bool(np.abs(d1v).max() > 0),
        'dw2': bool(np.abs(d2v).max() > 0),
    }
    key = (tuple(sorted(use_bias.items())), sq, sk, sv, so, s1, s2,
           tuple(sorted(MLP8)))
    if key not in _CACHE:
        _CACHE[key] = _build(use_bias, sq, sk, sv, so, s1, s2, MLP8)
    nc = _CACHE[key]
    l16 = sorted(set(range(L)) - MLP8)

    tri = np.tril(np.ones((128, 128), AF32)).T.astype(ABF16)
    ones_m = np.ones((128, 128), AF32).astype(ABF16)
    zeros_m = np.zeros((128, 128), AF32).astype(ABF16)
    shared = dict(
        emb_w=np.asarray(inputs['tok_emb_w'], AF32).astype(ABF16),
        emb_b=_r8(inputs['tok_emb_b']),
        wq=wq8, wk=wk8, wv=wv8, wo=wo8, w1=w18, w2=w28,
        bq=bqh, bk=bkh, bv=bvh, bo=boh, b1=b1h, b2=b2h,
        pw=pwf.astype(ABF16), pb=_r8(pv),
        dw1=np.asarray(inputs['dec_w1'], AF32).astype(ABF16), db1=_r8(d1v),
        dw2=dw2f.astype(ABF16), db2=np.ascontiguousarray(d2v.reshape(1, 128).T),
    )
    if l16:
        shared['w1b'] = w1f[l16].astype(ABF16)
        shared['w2b'] = w2f[l16].astype(ABF16)

    in_maps = []
    for c in range(8):
        b_, s = c // 2, c % 2
        tloc = np.concatenate([np.arange(128 * (2 * j + s), 128 * (2 * j + s) + 128)
                               for j in range(NB)])
        tnext = np.minimum(tloc + 1, T - 1)
        im = dict(shared)
        im['xloc'] = np.ascontiguousarray(x[b_][tloc].T).astype(ABF16)
        im['xshift'] = np.ascontiguousarray(x[b_][tnext].T)
        # parity-p key block j is attended by local query block jq when
        # global 2*jq+s >= 2*j+p; the diagonal jq==j needs tri when p==s,
        # everything when p<s (s=1,p=0), nothing when p>s (s=0,p=1).
        im['mask_a'] = tri if s == 0 else ones_m
        im['mask_b'] = zeros_m if s == 0 else tri
        in_maps.append(im)

    res = run_bass_kernel_spmd(nc, in_maps, list(range(8)))
    out = np.empty((B, T - 1, DIN), AF32)
    for c in range(8):
        b_, s = c // 2, c % 2
        o = res.results[c]['out'].T  # [TL, 128], DMA'd untransposed
        for j in range(NB):
            g = 2 * j + s
            t0, t1 = 128 * g, min(128 * g + 128, T - 1)
            out[b_, t0:t1] = o[128 * j:128 * j + (t1 - t0)]
    return out
